# revision 32
# baseline (speedup 1.0000x reference)
"""Bidirectional cross-attention Trainium2 kernel.

Data-parallel over batch B=8 across 8 NeuronCores (1 sample/core).

Per-core dataflow (matmul operands fp16, attention weights bf16, fp32 accum).

M-fold: S1 = Q1^T K2 = x1^T (Wq1^T Wk2) x2 + per-i + per-j + const bias terms.
  - M1 = Wq1^T Wk2 is precomputed on HOST; the device computes T2 = M1 x2
    (ONE projection replacing both the Q1 and K2 projections) and
    S1^T[j,i] = T2[:,j] . x1[:,i] with RAW x1 as the moving operand.
  - the per-i logit bias (Wq1^T bk2)^T x1 scales each softmax column by a
    constant -> cancels in softmax -> dropped.
  - the per-j logit bias u[j] = (Wk2^T bq1)^T x2 + bq1.bk2 becomes a factor
    g[j] = exp(u[j]) (computed on host, uploaded [128, NT] j-major) folded
    into the V tile and denominator column: V' = g*V, den-col = g.
  - the V-bias rides through softmax unchanged (sum_j p_j = 1): host folds it
    into the residual upload x1t_upload = x1^T + b_v2.
  expS1T[j,i] = exp(T2^T x1)  (ScalarE exp -> bf16; no max-subtract: logits are
                               bounded ~|41| for this problem's scale, exp fits fp32/bf16)
  outT[i, 0:257] = sum_j expS1T[j,i] * [g*V2T | g][j, :]
     -> col 256 is the softmax denominator; y = outT[:,0:256]/denominator + x1t_upload
  (symmetric for direction 2)
Chunks of 512 i-columns are software-pipelined: expS(k) matmuls+exps woven with
out(k-1) matmuls so PE never waits on ScalarE; dummy warm-up matmuls keep the
PE HAM clock at 2.4GHz during the input DMA window.

Host side: shard batch over cores, transpose weights/x, gather + transpose outputs.
"""

import sys

if "/opt/trn_rl_repo" not in sys.path:
    sys.path.insert(0, "/opt/trn_rl_repo")

import numpy as np

B, C, H, W = 8, 256, 48, 48
N = H * W  # 2304
NT = N // 128  # 18 j/i tiles
CT = C // 128  # 2 c tiles
CW = 512  # max i-chunk width for expS (last chunk is 256)
CHUNKS = [(0, 512), (512, 512), (1024, 512), (1536, 512), (2048, 256)]
# dir-2 ends with two 128-wide chunks: the final unwoven out-drain shrinks
# from 36 matmuls to 18 (shorter kernel tail)
CHUNKS2 = [(0, 512), (512, 512), (1024, 512), (1536, 512), (2048, 128), (2176, 128)]

_CACHE = {}


def _build():
    import concourse.bacc as bacc
    import concourse.mybir as mybir
    from concourse.tile import TileContext

    F32, F16, BF16 = mybir.dt.float32, mybir.dt.float16, mybir.dt.bfloat16
    Exp = mybir.ActivationFunctionType.Exp
    Ident = mybir.ActivationFunctionType.Identity

    nc = bacc.Bacc(None, target_bir_lowering=False)

    x_d = {
        "x1": nc.dram_tensor("x1", [C, N], F16, kind="ExternalInput"),
        "x2": nc.dram_tensor("x2", [C, N], F16, kind="ExternalInput"),
    }
    xt_d = {
        "x1t": nc.dram_tensor("x1t", [N, C], F16, kind="ExternalInput"),
        "x2t": nc.dram_tensor("x2t", [N, C], F16, kind="ExternalInput"),
    }
    w_names = ["m1t", "wv2t", "m2t", "wv1t"]  # pack order
    wpack_d = nc.dram_tensor("wpack", [C, 4 * C], F16, kind="ExternalInput")
    # g factors: [g1 | g2], each [128, NT] j-major (g[jt*128+p] at [p, jt])
    gpack_d = nc.dram_tensor("gpack", [128, 2 * NT], F32, kind="ExternalInput")
    y_d = {
        "y1t": nc.dram_tensor("y1t", [N, C], F16, kind="ExternalOutput"),
        "y2t": nc.dram_tensor("y2t", [N, C], F16, kind="ExternalOutput"),
    }

    with TileContext(nc) as tc:
        with (
            tc.tile_pool(name="const", bufs=1) as cp,
            tc.tile_pool(name="proj", bufs=1) as pp,
            tc.tile_pool(name="stream", bufs=4) as sp,
            tc.tile_pool(name="psum", bufs=2, space="PSUM") as psp,
            tc.tile_pool(name="psum_s", bufs=3, space="PSUM") as psp2,
        ):
            # ---------- setup: input loads + short PE warm-up ----------
            proj = {}
            # weights + first x1 halves go first so real projections can start
            # as soon as possible; a short warm-up covers DMA latency + the
            # PE clock ramp.
            wpack = cp.tile([128, CT, 4 * C], F16, tag="wpack")
            for ck in range(CT):
                nc.sync.dma_start(
                    out=wpack[:, ck, :], in_=wpack_d[ck * 128 : (ck + 1) * 128, :]
                )
            w_sb = {n: wpack[:, :, i * C : (i + 1) * C] for i, n in enumerate(w_names)}
            gt = cp.tile([128, 2, NT], F32, tag="gt")
            nc.sync.dma_start(out=gt[:, :, :], in_=gpack_d[:, :])

            x_sb = {}

            # dir-1 starts with T2 = M1 x2 and its proj consumes all of x2
            # within ~1.5us of starting, so x2 loads fully first with
            # max-size descriptors (full 4.6KB rows); x1 follows in halves
            # (finer dependencies for expS chunk 0 / T1 fill)
            xt1 = pp.tile([128, CT, N], F16, tag="x1")
            xt2 = pp.tile([128, CT, N], F16, tag="x2")
            x_sb["x1"], x_sb["x2"] = xt1, xt2
            for ck in range(CT):
                nc.sync.dma_start(
                    out=xt2[:, ck, :], in_=x_d["x2"][ck * 128 : (ck + 1) * 128, :]
                )
            for h0 in (0, N // 2):
                for ck in range(CT):
                    nc.sync.dma_start(
                        out=xt1[:, ck, h0 : h0 + N // 2],
                        in_=x_d["x1"][ck * 128 : (ck + 1) * 128, h0 : h0 + N // 2],
                    )

            # PE warm-up while input DMAs are in flight: keeps HAM at 8/8
            dummy = cp.tile([128, 512], F16, tag="warm")
            nc.vector.memset(dummy[:, :], 0.0)
            wps = None
            for _ in range(15):
                wps = psp.tile([128, 512], F32, tag="ps_o")
                nc.tensor.matmul(
                    wps[:, :], dummy[:, 0:128], dummy[:, :], start=True, stop=True
                )
            wexp = cp.tile([128, 512], F32, tag="warm_exp")
            nc.scalar.activation(wexp[:, :], wps[:, :], Exp)

            # ---------- projection action builders ----------
            def proj_t_actions(dst, xt, wn, alt0=0):
                # T = M x (no bias); copy psum->sbuf alternating ACT/DVE
                acts = []
                i = 0
                for ct in range(CT):
                    for c0, cw in CHUNKS:

                        def mk(ct, c0, cw, use_act):
                            def act():
                                ps2 = psp2.tile([128, 2, CW], F32, tag="ps_s")
                                ps = ps2[:, 0, :]
                                for ck in range(CT):
                                    nc.tensor.matmul(
                                        ps[:, 0:cw],
                                        w_sb[wn][:, ck, ct * 128 : (ct + 1) * 128],
                                        xt[:, ck, c0 : c0 + cw],
                                        start=(ck == 0),
                                        stop=(ck == CT - 1),
                                    )
                                if use_act:
                                    nc.scalar.activation(
                                        dst[:, ct, c0 : c0 + cw], ps[:, 0:cw], Ident
                                    )
                                else:
                                    nc.vector.tensor_copy(
                                        dst[:, ct, c0 : c0 + cw], ps[:, 0:cw]
                                    )

                            return act

                        acts.append(mk(ct, c0, cw, (alt0 + i) % 2 == 0))
                        i += 1
                return acts

            def proj_vt_actions(dst, xt, wn, gcol):
                # V' = g * (Wv x): per-partition g on the copy (DVE/ACT alternate)
                acts = []
                for jt in range(NT):

                    def mk(jt):
                        def act():
                            ps2 = psp2.tile([128, 2, CW], F32, tag="ps_s")
                            ps = ps2[:, 0, :]
                            for ck in range(CT):
                                nc.tensor.matmul(
                                    ps[:, 0:C],
                                    xt[:, ck, jt * 128 : (jt + 1) * 128],
                                    w_sb[wn][:, ck, :],
                                    start=(ck == 0),
                                    stop=(ck == CT - 1),
                                )
                            if jt % 2 == 0:
                                nc.vector.tensor_scalar_mul(
                                    dst[:, jt, 0:C], ps[:, 0:C], gcol[:, jt : jt + 1]
                                )
                            else:
                                nc.scalar.activation(
                                    dst[:, jt, 0:C],
                                    ps[:, 0:C],
                                    Ident,
                                    scale=gcol[:, jt : jt + 1],
                                )

                        return act

                    acts.append(mk(jt))
                return acts

            for nm in ["T2", "T1"]:
                proj[nm] = pp.tile([128, CT, N], F16, tag=nm, name=nm)
            for d, nm in enumerate(["VT1", "VT2"]):
                # note: g index d: gt[:, 0, :]=g1 scales VT2 (dir 1 uses V2),
                # gt[:, 1, :]=g2 scales VT1
                proj[nm] = pp.tile([128, NT, C + 1], BF16, tag=nm, name=nm)
                # denominator column (col C) = g; V writes cover 0:C
                nc.vector.tensor_copy(proj[nm][:, :, C], gt[:, 1 - d, :])

            # only T2 must precede dir-1 attention; VT2 is consumed by
            # out(c0) whose emission starts in chunk 1, so VT2 and all dir-2
            # projections become fill work woven into dir-1's attention chunks
            for a in proj_t_actions(proj["T2"], x_sb["x2"], "m1t", 0):
                a()
            vt2_acts = proj_vt_actions(proj["VT2"], x_sb["x2"], "wv2t", gt[:, 0, :])
            fill = (
                vt2_acts
                + proj_t_actions(proj["T1"], x_sb["x1"], "m2t", 1)
                + proj_vt_actions(proj["VT1"], x_sb["x1"], "wv1t", gt[:, 1, :])
            )
            # per-chunk fill quotas: ALL of VT2 must be emitted within chunk 0;
            # only +2 T1 actions there (the 3rd reads x1's second half, which
            # is still in flight during chunk 0)
            n_vt2 = len(vt2_acts)
            rest = len(fill) - n_vt2 - 2
            quotas = [n_vt2 + 2] + [(rest + 3) // 4] * 4

            # ---------- attention ----------
            with tc.tile_pool(name="ep", bufs=2) as ep:

                def exp_actions(Q, K, e, c0, cw):
                    # one action = expS matmuls + one wide exp for a PAIR of j-tiles
                    def mk(jp):
                        def act():
                            ps2 = psp2.tile([128, 2, CW], F32, tag="ps_s")
                            for jj in range(2):
                                jt = jp + jj
                                for ck in range(CT):
                                    nc.tensor.matmul(
                                        ps2[:, jj, 0:cw],
                                        K[:, ck, jt * 128 : (jt + 1) * 128],
                                        Q[:, ck, c0 : c0 + cw],
                                        start=(ck == 0),
                                        stop=(ck == CT - 1),
                                    )
                            nc.scalar.activation(
                                e[:, jp : jp + 2, 0:cw], ps2[:, :, 0:cw], Exp
                            )

                        return act

                    return [mk(jp) for jp in range(0, NT, 2)]

                def out_actions(e, VT, xt_dram, yt_dram, c0, cw):
                    # actions = out-matmul slices + epilogue, per i-subtile
                    acts = []
                    for il in range(cw // 128):
                        it = c0 // 128 + il
                        po = psp.tile([128, C + 1], F32, tag="ps_o")

                        xt_t = sp.tile([128, C], F16, tag="xt")

                        def mk_mm(po, il, it, j0, jn, xt_t):
                            def act():
                                # residual load rides mid-chain: late enough to
                                # stay off the startup DMA burst, early enough
                                # for the epilogue
                                if j0 == 5:
                                    nc.sync.dma_start(
                                        out=xt_t[:, :],
                                        in_=xt_dram[it * 128 : (it + 1) * 128, :],
                                    )
                                for jt in range(j0, jn):
                                    nc.tensor.matmul(
                                        po[:, :],
                                        e[:, jt, il * 128 : (il + 1) * 128],
                                        VT[:, jt, :],
                                        start=(jt == 0),
                                        stop=(jt == NT - 1),
                                    )

                            return act

                        for j0 in range(0, NT, 5):
                            acts.append(mk_mm(po, il, it, j0, min(j0 + 5, NT), xt_t))

                        def mk_epi(po, it, xt_t):
                            def act():
                                r = sp.tile([128, 1], F32, tag="r")
                                nc.vector.reciprocal(r[:, :], po[:, C : C + 1])
                                y = sp.tile([128, C], F16, tag="y")
                                nc.vector.scalar_tensor_tensor(
                                    y[:, :],
                                    po[:, 0:C],
                                    r[:, :],
                                    xt_t[:, :],
                                    op0=mybir.AluOpType.mult,
                                    op1=mybir.AluOpType.add,
                                )
                                nc.sync.dma_start(
                                    out=yt_dram[it * 128 : (it + 1) * 128, :], in_=y[:, :]
                                )

                            return act

                        acts.append(mk_epi(po, it, xt_t))
                    return acts

                def weave(a, b):
                    # emit all of a and b interleaved evenly (a paces, b fills)
                    if not b:
                        for f in a:
                            f()
                        return
                    na, nb = len(a), len(b)
                    j = 0
                    for i, f in enumerate(a):
                        f()
                        while j < nb and j * na <= (i + 1) * nb - 1:
                            b[j]()
                            j += 1
                    while j < nb:
                        b[j]()
                        j += 1

                # software pipeline: expS(k) woven with out(k-1); dir-2 projections
                # are distributed as extra fill across dir-1's chunks (they MUST
                # all be emitted before dir-2's first expS reads Q2/K1/VT1)
                plan = [
                    (x_sb["x1"], proj["T2"], proj["VT2"], xt_d["x1t"], y_d["y1t"], c0, cw)
                    for c0, cw in CHUNKS
                ] + [
                    (x_sb["x2"], proj["T1"], proj["VT1"], xt_d["x2t"], y_d["y2t"], c0, cw)
                    for c0, cw in CHUNKS2
                ]
                nd1 = len(CHUNKS)
                pending = []
                for step, (Q, K, VT, xtd, ytd, c0, cw) in enumerate(plan):
                    if step < nd1:
                        q = quotas[step]
                        extra, fill = fill[:q], fill[q:]
                    else:
                        assert not fill
                        extra = []
                    if step == 0:
                        # a few VT2 fills ahead of the first exp: x1 (the exp
                        # moving operand) is still in flight right after T2
                        for f in extra[:6]:
                            f()
                        extra = extra[6:]
                    e = ep.tile([128, NT, CW], BF16, tag="e")
                    weave(exp_actions(Q, K, e, c0, cw), pending + extra)
                    pending = out_actions(e, VT, xtd, ytd, c0, cw)
                weave(pending, [])

    nc.compile()
    return nc


def _get_nc():
    if "nc" not in _CACHE:
        _CACHE["nc"] = _build()
    return _CACHE["nc"]


def kernel(
    x1,
    x2,
    w_q1,
    b_q1,
    w_k1,
    b_k1,
    w_v1,
    b_v1,
    w_q2,
    b_q2,
    w_k2,
    b_k2,
    w_v2,
    b_v2,
    _trace=False,
):
    from concourse.bass_utils import run_bass_kernel_spmd

    nc = _get_nc()

    x1 = np.asarray(x1, dtype=np.float32)
    x2 = np.asarray(x2, dtype=np.float32)
    x1h = x1.astype(np.float16)
    x2h = x2.astype(np.float16)
    w_q1, w_k1, w_v1 = (np.asarray(w, np.float32) for w in (w_q1, w_k1, w_v1))
    w_q2, w_k2, w_v2 = (np.asarray(w, np.float32) for w in (w_q2, w_k2, w_v2))
    b_q1, b_k1, b_q2, b_k2 = (
        np.asarray(b, np.float32) for b in (b_q1, b_k1, b_q2, b_k2)
    )
    # M-fold: M1 = wq1^T wk2, M2 = wq2^T wk1; pack M^T as the lhsT weights
    m1t = w_k2.T @ w_q1
    m2t = w_k1.T @ w_q2
    # wpack order must match w_names: m1t, wv2t, m2t, wv1t
    wpack = np.ascontiguousarray(
        np.concatenate([m1t, w_v2.T, m2t, w_v1.T], axis=1).astype(np.float16)
    )
    # per-j softmax factor g[j] = exp((Wk^T bq)^T x + bq.bk)
    q1v = w_k2.T @ b_q1
    c1 = float(b_q1 @ b_k2)
    q2v = w_k1.T @ b_q2
    c2 = float(b_q2 @ b_k1)
    # V-bias passes through softmax unchanged -> folded into the residual term
    bv1 = np.asarray(b_v1, np.float32).reshape(1, C)
    bv2 = np.asarray(b_v2, np.float32).reshape(1, C)

    in_maps = []
    for i in range(B):
        x1i = np.ascontiguousarray(x1[i].reshape(C, N))
        x2i = np.ascontiguousarray(x2[i].reshape(C, N))
        g1 = np.exp(q1v @ x2i + c1).reshape(N // 128, 128).T
        g2 = np.exp(q2v @ x1i + c2).reshape(N // 128, 128).T
        m = {
            "x1": np.ascontiguousarray(x1h[i].reshape(C, N)),
            "x2": np.ascontiguousarray(x2h[i].reshape(C, N)),
            "x1t": np.ascontiguousarray((x1i.T + bv2).astype(np.float16)),
            "x2t": np.ascontiguousarray((x2i.T + bv1).astype(np.float16)),
            "wpack": wpack,
            "gpack": np.ascontiguousarray(
                np.concatenate([g1, g2], axis=1).astype(np.float32)
            ),
        }
        in_maps.append(m)

    res = run_bass_kernel_spmd(nc, in_maps, list(range(B)), trace=_trace)
    if _trace:
        _CACHE["last_result"] = res

    y1 = np.empty((B, C, H, W), np.float32)
    y2 = np.empty((B, C, H, W), np.float32)
    for i in range(B):
        y1[i] = res.results[i]["y1t"].astype(np.float32).T.reshape(C, H, W)
        y2[i] = res.results[i]["y2t"].astype(np.float32).T.reshape(C, H, W)
    return y1, y2



# revision 34
# speedup vs baseline: 1.0048x; 1.0048x over previous
"""Bidirectional cross-attention Trainium2 kernel.

Data-parallel over batch B=8 across 8 NeuronCores (1 sample/core).

Per-core dataflow (matmul operands fp16, attention weights bf16, fp32 accum).

M-fold: S1 = Q1^T K2 = x1^T (Wq1^T Wk2) x2 + per-i + per-j + const bias terms.
  - M1 = Wq1^T Wk2 is precomputed on HOST; the device computes T2 = M1 x2
    (ONE projection replacing both the Q1 and K2 projections) and
    S1^T[j,i] = T2[:,j] . x1[:,i] with RAW x1 as the moving operand.
  - the per-i logit bias (Wq1^T bk2)^T x1 scales each softmax column by a
    constant -> cancels in softmax -> dropped.
  - the per-j logit bias u[j] = (Wk2^T bq1)^T x2 + bq1.bk2 becomes a factor
    g[j] = exp(u[j]) (computed on host, uploaded [128, NT] j-major) folded
    into the V tile and denominator column: V' = g*V, den-col = g.
  - the V-bias rides through softmax unchanged (sum_j p_j = 1): host folds it
    into the residual upload x1t_upload = x1^T + b_v2.
  expS1T[j,i] = exp(T2^T x1)  (ScalarE exp -> bf16; no max-subtract: logits are
                               bounded ~|41| for this problem's scale, exp fits fp32/bf16)
  outT[i, 0:257] = sum_j expS1T[j,i] * [g*V2T | g][j, :]
     -> col 256 is the softmax denominator; y = outT[:,0:256]/denominator + x1t_upload
  (symmetric for direction 2)
Chunks of 512 i-columns are software-pipelined: expS(k) matmuls+exps woven with
out(k-1) matmuls so PE never waits on ScalarE; dummy warm-up matmuls keep the
PE HAM clock at 2.4GHz during the input DMA window.

Host side: shard batch over cores, transpose weights/x, gather + transpose outputs.
"""

import sys

if "/opt/trn_rl_repo" not in sys.path:
    sys.path.insert(0, "/opt/trn_rl_repo")

import numpy as np

B, C, H, W = 8, 256, 48, 48
N = H * W  # 2304
NT = N // 128  # 18 j/i tiles
CT = C // 128  # 2 c tiles
CW = 512  # max i-chunk width for expS (last chunk is 256)
CHUNKS = [(0, 512), (512, 512), (1024, 512), (1536, 512), (2048, 256)]
# dir-2 ends with two 128-wide chunks: the final unwoven out-drain shrinks
# from 36 matmuls to 18 (shorter kernel tail)
CHUNKS2 = [(0, 512), (512, 512), (1024, 512), (1536, 512), (2048, 128), (2176, 128)]

_CACHE = {}


def _build():
    import concourse.bacc as bacc
    import concourse.mybir as mybir
    from concourse.tile import TileContext

    F32, F16, BF16 = mybir.dt.float32, mybir.dt.float16, mybir.dt.bfloat16
    Exp = mybir.ActivationFunctionType.Exp
    Ident = mybir.ActivationFunctionType.Identity

    nc = bacc.Bacc(None, target_bir_lowering=False)

    x_d = {
        "x1": nc.dram_tensor("x1", [C, N], F16, kind="ExternalInput"),
        "x2": nc.dram_tensor("x2", [C, N], F16, kind="ExternalInput"),
    }
    xt_d = {
        "x1t": nc.dram_tensor("x1t", [N, C], F16, kind="ExternalInput"),
        "x2t": nc.dram_tensor("x2t", [N, C], F16, kind="ExternalInput"),
    }
    w_names = ["m1t", "wv2t", "m2t", "wv1t"]  # pack order
    wpack_d = nc.dram_tensor("wpack", [C, 4 * C], F16, kind="ExternalInput")
    # g factors: [g1 | g2], each [128, NT] j-major (g[jt*128+p] at [p, jt])
    gpack_d = nc.dram_tensor("gpack", [128, 2 * NT], F32, kind="ExternalInput")
    y_d = {
        "y1t": nc.dram_tensor("y1t", [N, C], F16, kind="ExternalOutput"),
        "y2t": nc.dram_tensor("y2t", [N, C], F16, kind="ExternalOutput"),
    }

    with TileContext(nc) as tc:
        with (
            tc.tile_pool(name="const", bufs=1) as cp,
            tc.tile_pool(name="proj", bufs=1) as pp,
            tc.tile_pool(name="stream", bufs=4) as sp,
            tc.tile_pool(name="psum", bufs=2, space="PSUM") as psp,
            tc.tile_pool(name="psum_s", bufs=3, space="PSUM") as psp2,
        ):
            # ---------- setup: input loads + short PE warm-up ----------
            proj = {}
            # weights + first x1 halves go first so real projections can start
            # as soon as possible; a short warm-up covers DMA latency + the
            # PE clock ramp.
            wpack = cp.tile([128, CT, 4 * C], F16, tag="wpack")
            for ck in range(CT):
                nc.sync.dma_start(
                    out=wpack[:, ck, :], in_=wpack_d[ck * 128 : (ck + 1) * 128, :]
                )
            w_sb = {n: wpack[:, :, i * C : (i + 1) * C] for i, n in enumerate(w_names)}
            gt = cp.tile([128, 2, NT], F32, tag="gt")
            nc.sync.dma_start(out=gt[:, :, :], in_=gpack_d[:, :])

            x_sb = {}

            # dir-1 starts with T2 = M1 x2 and its proj consumes all of x2
            # within ~1.5us of starting, so x2 loads fully first with
            # max-size descriptors (full 4.6KB rows); x1 follows in halves
            # (finer dependencies for expS chunk 0 / T1 fill)
            xt1 = pp.tile([128, CT, N], F16, tag="x1")
            xt2 = pp.tile([128, CT, N], F16, tag="x2")
            x_sb["x1"], x_sb["x2"] = xt1, xt2
            for ck in range(CT):
                nc.sync.dma_start(
                    out=xt2[:, ck, :], in_=x_d["x2"][ck * 128 : (ck + 1) * 128, :]
                )
            for h0 in (0, N // 2):
                for ck in range(CT):
                    nc.sync.dma_start(
                        out=xt1[:, ck, h0 : h0 + N // 2],
                        in_=x_d["x1"][ck * 128 : (ck + 1) * 128, h0 : h0 + N // 2],
                    )

            # PE warm-up while input DMAs are in flight: keeps HAM at 8/8
            dummy = cp.tile([128, 512], F16, tag="warm")
            nc.vector.memset(dummy[:, :], 0.0)
            wps = None
            for _ in range(20):
                wps = psp.tile([128, 512], F32, tag="ps_o")
                nc.tensor.matmul(
                    wps[:, :], dummy[:, 0:128], dummy[:, :], start=True, stop=True
                )
            wexp = cp.tile([128, 512], F32, tag="warm_exp")
            nc.scalar.activation(wexp[:, :], wps[:, :], Exp)

            # ---------- projection action builders ----------
            def proj_t_actions(dst, xt, wn, alt0=0):
                # T = M x (no bias); copy psum->sbuf alternating ACT/DVE
                acts = []
                i = 0
                for ct in range(CT):
                    for c0, cw in CHUNKS:

                        def mk(ct, c0, cw, use_act):
                            def act():
                                ps2 = psp2.tile([128, 2, CW], F32, tag="ps_s")
                                ps = ps2[:, 0, :]
                                for ck in range(CT):
                                    nc.tensor.matmul(
                                        ps[:, 0:cw],
                                        w_sb[wn][:, ck, ct * 128 : (ct + 1) * 128],
                                        xt[:, ck, c0 : c0 + cw],
                                        start=(ck == 0),
                                        stop=(ck == CT - 1),
                                    )
                                if use_act:
                                    nc.scalar.activation(
                                        dst[:, ct, c0 : c0 + cw], ps[:, 0:cw], Ident
                                    )
                                else:
                                    nc.vector.tensor_copy(
                                        dst[:, ct, c0 : c0 + cw], ps[:, 0:cw]
                                    )

                            return act

                        acts.append(mk(ct, c0, cw, (alt0 + i) % 2 == 0))
                        i += 1
                return acts

            def proj_vt_actions(dst, xt, wn, gcol):
                # V' = g * (Wv x): per-partition g on the copy (DVE/ACT alternate)
                acts = []
                for jt in range(NT):

                    def mk(jt):
                        def act():
                            ps2 = psp2.tile([128, 2, CW], F32, tag="ps_s")
                            ps = ps2[:, 0, :]
                            for ck in range(CT):
                                nc.tensor.matmul(
                                    ps[:, 0:C],
                                    xt[:, ck, jt * 128 : (jt + 1) * 128],
                                    w_sb[wn][:, ck, :],
                                    start=(ck == 0),
                                    stop=(ck == CT - 1),
                                )
                            if jt % 2 == 0:
                                nc.vector.tensor_scalar_mul(
                                    dst[:, jt, 0:C], ps[:, 0:C], gcol[:, jt : jt + 1]
                                )
                            else:
                                nc.scalar.activation(
                                    dst[:, jt, 0:C],
                                    ps[:, 0:C],
                                    Ident,
                                    scale=gcol[:, jt : jt + 1],
                                )

                        return act

                    acts.append(mk(jt))
                return acts

            for nm in ["T2", "T1"]:
                proj[nm] = pp.tile([128, CT, N], F16, tag=nm, name=nm)
            for d, nm in enumerate(["VT1", "VT2"]):
                # note: g index d: gt[:, 0, :]=g1 scales VT2 (dir 1 uses V2),
                # gt[:, 1, :]=g2 scales VT1
                proj[nm] = pp.tile([128, NT, C + 1], BF16, tag=nm, name=nm)
                # denominator column (col C) = g; V writes cover 0:C
                nc.vector.tensor_copy(proj[nm][:, :, C], gt[:, 1 - d, :])

            # only T2 must precede dir-1 attention; VT2 is consumed by
            # out(c0) whose emission starts in chunk 1, so VT2 and all dir-2
            # projections become fill work woven into dir-1's attention chunks
            for a in proj_t_actions(proj["T2"], x_sb["x2"], "m1t", 0):
                a()
            vt2_acts = proj_vt_actions(proj["VT2"], x_sb["x2"], "wv2t", gt[:, 0, :])
            fill = (
                vt2_acts
                + proj_t_actions(proj["T1"], x_sb["x1"], "m2t", 1)
                + proj_vt_actions(proj["VT1"], x_sb["x1"], "wv1t", gt[:, 1, :])
            )
            # per-chunk fill quotas: ALL of VT2 must be emitted within chunk 0;
            # only +2 T1 actions there (the 3rd reads x1's second half, which
            # is still in flight during chunk 0)
            n_vt2 = len(vt2_acts)
            rest = len(fill) - n_vt2 - 2
            quotas = [n_vt2 + 2] + [(rest + 3) // 4] * 4

            # ---------- attention ----------
            with tc.tile_pool(name="ep", bufs=2) as ep:

                def exp_actions(Q, K, e, c0, cw):
                    # one action = expS matmuls + one wide exp for a PAIR of j-tiles
                    def mk(jp):
                        def act():
                            ps2 = psp2.tile([128, 2, CW], F32, tag="ps_s")
                            for jj in range(2):
                                jt = jp + jj
                                for ck in range(CT):
                                    nc.tensor.matmul(
                                        ps2[:, jj, 0:cw],
                                        K[:, ck, jt * 128 : (jt + 1) * 128],
                                        Q[:, ck, c0 : c0 + cw],
                                        start=(ck == 0),
                                        stop=(ck == CT - 1),
                                    )
                            nc.scalar.activation(
                                e[:, jp : jp + 2, 0:cw], ps2[:, :, 0:cw], Exp
                            )

                        return act

                    return [mk(jp) for jp in range(0, NT, 2)]

                def out_actions(e, VT, xt_dram, yt_dram, c0, cw):
                    # actions = out-matmul slices + epilogue, per i-subtile
                    acts = []
                    for il in range(cw // 128):
                        it = c0 // 128 + il
                        po = psp.tile([128, C + 1], F32, tag="ps_o")

                        xt_t = sp.tile([128, C], F16, tag="xt")

                        def mk_mm(po, il, it, j0, jn, xt_t):
                            def act():
                                # residual load rides mid-chain: late enough to
                                # stay off the startup DMA burst, early enough
                                # for the epilogue
                                if j0 == 5:
                                    nc.sync.dma_start(
                                        out=xt_t[:, :],
                                        in_=xt_dram[it * 128 : (it + 1) * 128, :],
                                    )
                                for jt in range(j0, jn):
                                    nc.tensor.matmul(
                                        po[:, :],
                                        e[:, jt, il * 128 : (il + 1) * 128],
                                        VT[:, jt, :],
                                        start=(jt == 0),
                                        stop=(jt == NT - 1),
                                    )

                            return act

                        for j0 in range(0, NT, 5):
                            acts.append(mk_mm(po, il, it, j0, min(j0 + 5, NT), xt_t))

                        def mk_epi(po, it, xt_t):
                            def act():
                                r = sp.tile([128, 1], F32, tag="r")
                                nc.vector.reciprocal(r[:, :], po[:, C : C + 1])
                                y = sp.tile([128, C], F16, tag="y")
                                nc.vector.scalar_tensor_tensor(
                                    y[:, :],
                                    po[:, 0:C],
                                    r[:, :],
                                    xt_t[:, :],
                                    op0=mybir.AluOpType.mult,
                                    op1=mybir.AluOpType.add,
                                )
                                nc.sync.dma_start(
                                    out=yt_dram[it * 128 : (it + 1) * 128, :], in_=y[:, :]
                                )

                            return act

                        acts.append(mk_epi(po, it, xt_t))
                    return acts

                def weave(a, b):
                    # emit all of a and b interleaved evenly (a paces, b fills)
                    if not b:
                        for f in a:
                            f()
                        return
                    na, nb = len(a), len(b)
                    j = 0
                    for i, f in enumerate(a):
                        f()
                        while j < nb and j * na <= (i + 1) * nb - 1:
                            b[j]()
                            j += 1
                    while j < nb:
                        b[j]()
                        j += 1

                # software pipeline: expS(k) woven with out(k-1); dir-2 projections
                # are distributed as extra fill across dir-1's chunks (they MUST
                # all be emitted before dir-2's first expS reads Q2/K1/VT1)
                plan = [
                    (x_sb["x1"], proj["T2"], proj["VT2"], xt_d["x1t"], y_d["y1t"], c0, cw)
                    for c0, cw in CHUNKS
                ] + [
                    (x_sb["x2"], proj["T1"], proj["VT1"], xt_d["x2t"], y_d["y2t"], c0, cw)
                    for c0, cw in CHUNKS2
                ]
                nd1 = len(CHUNKS)
                pending = []
                for step, (Q, K, VT, xtd, ytd, c0, cw) in enumerate(plan):
                    if step < nd1:
                        q = quotas[step]
                        extra, fill = fill[:q], fill[q:]
                    else:
                        assert not fill
                        extra = []
                    if step == 0:
                        # a few VT2 fills ahead of the first exp: x1 (the exp
                        # moving operand) is still in flight right after T2
                        for f in extra[:8]:
                            f()
                        extra = extra[8:]
                    e = ep.tile([128, NT, CW], BF16, tag="e")
                    weave(exp_actions(Q, K, e, c0, cw), pending + extra)
                    pending = out_actions(e, VT, xtd, ytd, c0, cw)
                weave(pending, [])

    nc.compile()
    return nc


def _get_nc():
    if "nc" not in _CACHE:
        _CACHE["nc"] = _build()
    return _CACHE["nc"]


def kernel(
    x1,
    x2,
    w_q1,
    b_q1,
    w_k1,
    b_k1,
    w_v1,
    b_v1,
    w_q2,
    b_q2,
    w_k2,
    b_k2,
    w_v2,
    b_v2,
    _trace=False,
):
    from concourse.bass_utils import run_bass_kernel_spmd

    nc = _get_nc()

    x1 = np.asarray(x1, dtype=np.float32)
    x2 = np.asarray(x2, dtype=np.float32)
    x1h = x1.astype(np.float16)
    x2h = x2.astype(np.float16)
    w_q1, w_k1, w_v1 = (np.asarray(w, np.float32) for w in (w_q1, w_k1, w_v1))
    w_q2, w_k2, w_v2 = (np.asarray(w, np.float32) for w in (w_q2, w_k2, w_v2))
    b_q1, b_k1, b_q2, b_k2 = (
        np.asarray(b, np.float32) for b in (b_q1, b_k1, b_q2, b_k2)
    )
    # M-fold: M1 = wq1^T wk2, M2 = wq2^T wk1; pack M^T as the lhsT weights
    m1t = w_k2.T @ w_q1
    m2t = w_k1.T @ w_q2
    # wpack order must match w_names: m1t, wv2t, m2t, wv1t
    wpack = np.ascontiguousarray(
        np.concatenate([m1t, w_v2.T, m2t, w_v1.T], axis=1).astype(np.float16)
    )
    # per-j softmax factor g[j] = exp((Wk^T bq)^T x + bq.bk)
    q1v = w_k2.T @ b_q1
    c1 = float(b_q1 @ b_k2)
    q2v = w_k1.T @ b_q2
    c2 = float(b_q2 @ b_k1)
    # V-bias passes through softmax unchanged -> folded into the residual term
    bv1 = np.asarray(b_v1, np.float32).reshape(1, C)
    bv2 = np.asarray(b_v2, np.float32).reshape(1, C)

    in_maps = []
    for i in range(B):
        x1i = np.ascontiguousarray(x1[i].reshape(C, N))
        x2i = np.ascontiguousarray(x2[i].reshape(C, N))
        g1 = np.exp(q1v @ x2i + c1).reshape(N // 128, 128).T
        g2 = np.exp(q2v @ x1i + c2).reshape(N // 128, 128).T
        m = {
            "x1": np.ascontiguousarray(x1h[i].reshape(C, N)),
            "x2": np.ascontiguousarray(x2h[i].reshape(C, N)),
            "x1t": np.ascontiguousarray((x1i.T + bv2).astype(np.float16)),
            "x2t": np.ascontiguousarray((x2i.T + bv1).astype(np.float16)),
            "wpack": wpack,
            "gpack": np.ascontiguousarray(
                np.concatenate([g1, g2], axis=1).astype(np.float32)
            ),
        }
        in_maps.append(m)

    res = run_bass_kernel_spmd(nc, in_maps, list(range(B)), trace=_trace)
    if _trace:
        _CACHE["last_result"] = res

    y1 = np.empty((B, C, H, W), np.float32)
    y2 = np.empty((B, C, H, W), np.float32)
    for i in range(B):
        y1[i] = res.results[i]["y1t"].astype(np.float32).T.reshape(C, H, W)
        y2[i] = res.results[i]["y2t"].astype(np.float32).T.reshape(C, H, W)
    return y1, y2

